# revision 47
# baseline (speedup 1.0000x reference)
"""Trainium2 Bass kernel for ComplexResNet: 8-core data-parallel, bf16.

Layout: features on partitions, samples on matmul free dim (512/tile).
Convs = dense W_eff blocks over a packed feature space (zero blocks
skipped), biases via ACT-bias / scalar_tensor_tensor. MaxPool = elementwise
max between even/odd parity chunks. Head: sigmoid -> arctan(si/sr) via
reciprocal+mul (args positive so no quadrant fix), block-diagonal FC over
6-tile groups.

x is loaded with partition p <- samples 4p..4p+3 (contiguous 1056B DMA
lines); the resulting within-tile sample permutation (col u*128+j <->
sample 4j+u) is undone by the output DMA access pattern.
"""
import math
import numpy as np

B = 262144
NCORES = 8
BC = B // NCORES          # 32768 samples per core
NT = 512                  # samples per tile
NTILES = BC // NT         # 64
GROUPS = [(g, 4) for g in range(0, NTILES, 4)]


# ---------------------------------------------------------------------------
# Host-side W_eff construction
# ---------------------------------------------------------------------------
def _conv_weff(wr, wi, Lin, Lout, pad, fin, fout):
    """Stacked-complex conv as dense real matrix W[fout_dim, fin_dim].
    cross-correlation: xin position li = lo + k - pad.
    fin(s, c, l)->col or None; fout: list of (s, c, l) rows."""
    Co, Ci, K = wr.shape
    nin = max(v for v in (fin(s, c, l) for s in range(2) for c in range(Ci)
                          for l in range(Lin)) if v is not None) + 1
    W = np.zeros((len(fout), nin), dtype=np.float64)
    for row, (so, co, lo) in enumerate(fout):
        for ci in range(Ci):
            for k in range(K):
                li = lo + k - pad
                if li < 0 or li >= Lin:
                    continue
                c0 = fin(0, ci, li)
                c1 = fin(1, ci, li)
                if so == 0:  # real out: wr*xr - wi*xi
                    if c0 is not None:
                        W[row, c0] += wr[co, ci, k]
                    if c1 is not None:
                        W[row, c1] -= wi[co, ci, k]
                else:        # imag out: wi*xr + wr*xi
                    if c0 is not None:
                        W[row, c0] += wi[co, ci, k]
                    if c1 is not None:
                        W[row, c1] += wr[co, ci, k]
    return W.astype(np.float32)


def _build_host(inp):
    """Returns dict of weight blocks / bias columns for the bass kernel."""
    g = lambda n: np.asarray(inp[n], dtype=np.float32)

    # x: f = s*33 + l
    fin_x = lambda s, c, l: s * 33 + l
    # a1: 5 K-tiles of 8 pos: f = (l//8)*128 + (l%8)*16 + s*8 + c
    def fin_a1(s, c, l):
        return (l // 8) * 128 + (l % 8) * 16 + s * 8 + c
    # res1 out rows (pos 0..31), chunk order e0,e1,o0,o1 each 128 rows
    def rows_r1(par, half):
        out = []
        for pl in range(8):
            p = 2 * (half * 8 + pl) + par
            for s in range(2):
                for c in range(8):
                    out.append((s, c, p))
        return out
    # p1: f = (lp//8)*128 + (lp%8)*16 + s*8 + c  (pooled pos lp 0..15)
    def fin_p1(s, c, lp):
        return (lp // 8) * 128 + (lp % 8) * 16 + s * 8 + c
    # a3: f = p*8 + s*4 + c   (p 0..15)
    fin_a3 = lambda s, c, p: p * 8 + s * 4 + c
    # res2 out rows, parity-merged: even pos rows 0-63, odd rows 64-127
    def rows_r2(par):
        out = []
        for pl in range(8):
            p = 2 * pl + par
            for s in range(2):
                for c in range(4):
                    out.append((s, c, p))
        return out

    W = {}
    bias = {}

    # ---- L1: x -> a1 (r1c1), 5 M-chunks (a1 feature order)
    fout_a1 = [None] * 528
    for l in range(33):
        for s in range(2):
            for c in range(8):
                fout_a1[fin_a1(s, c, l)] = (s, c, l)
    W1 = _conv_weff(g('r1c1_wr'), g('r1c1_wi'), 33, 33, 1, fin_x, fout_a1)
    W['L1'] = [W1[k * 128:(k + 1) * 128] for k in range(4)] + [W1[512:528]]
    bias['b1'] = np.tile(np.concatenate([g('r1c1_br'), g('r1c1_bi')]), 8)

    # ---- L2: a1 -> r1 conv2 out, M-chunks e0,e1,o0,o1 ; K-tiles = 5 a1 tiles
    r1_rows = [rows_r1(0, 0), rows_r1(0, 1), rows_r1(1, 0), rows_r1(1, 1)]
    W2_full = [_conv_weff(g('r1c2_wr'), g('r1c2_wi'), 33, 33, 1, fin_a1, rows)
               for rows in r1_rows]
    ksl = [(0, 128), (128, 256), (256, 384), (384, 512), (512, 528)]
    W['L2'] = [[Wm[:, a:b] for (a, b) in ksl] for Wm in W2_full]
    bias['b2'] = np.tile(np.concatenate([g('r1c2_br'), g('r1c2_bi')]), 8)

    # ---- SC1: x -> r1 shortcut (1x1), same M-chunks
    W['SC1'] = [_conv_weff(g('r1sc_wr'), g('r1sc_wi'), 33, 33, 0, fin_x, rows)
                for rows in r1_rows]
    bias['bsc1'] = np.tile(np.concatenate([g('r1sc_br'), g('r1sc_bi')]), 8)

    # ---- L3: p1 -> a3 (r2c1), M = 128, K-tiles = 2 p1 tiles
    fout_a3 = [None] * 128
    for p in range(16):
        for s in range(2):
            for c in range(4):
                fout_a3[fin_a3(s, c, p)] = (s, c, p)
    W3 = _conv_weff(g('r2c1_wr'), g('r2c1_wi'), 16, 16, 1, fin_p1, fout_a3)
    W['L3'] = [W3[:, 0:128], W3[:, 128:256]]
    bias['b3'] = np.tile(np.concatenate([g('r2c1_br'), g('r2c1_bi')]), 16)[:128]

    # ---- L4: a3 -> r2 conv2 out, M-chunks even/odd [64], K = 128
    W['L4'] = [_conv_weff(g('r2c2_wr'), g('r2c2_wi'), 16, 16, 1, fin_a3,
                          rows_r2(m)) for m in range(2)]
    bias['b4'] = np.tile(np.concatenate([g('r2c2_br'), g('r2c2_bi')]), 8)[:64]

    # ---- SC2: p1 -> r2 shortcut, even/odd chunks, K = 256 (2 tiles)
    W['SC2'] = [[Wm[:, 0:128], Wm[:, 128:256]] for Wm in
                (_conv_weff(g('r2sc_wr'), g('r2sc_wi'), 16, 16, 0, fin_p1,
                            rows_r2(m)) for m in range(2))]
    bias['bsc2'] = np.tile(np.concatenate([g('r2sc_br'), g('r2sc_bi')]), 8)[:64]

    # ---- La: p2 -> lr(20) / li(20)  (p2 flat idx = p*8 + s*4 + c)
    la_wr, la_wi = g('la_wr'), g('la_wi')  # [20, 32] torch-flat idx c*8+p
    Wla = np.zeros((40, 64), dtype=np.float32)
    for j in range(20):
        for c in range(4):
            for p in range(8):
                Wla[j, p * 8 + c] = la_wr[j, c * 8 + p]
                Wla[j, p * 8 + 4 + c] = -la_wi[j, c * 8 + p]
                Wla[20 + j, p * 8 + c] = la_wi[j, c * 8 + p]
                Wla[20 + j, p * 8 + 4 + c] = la_wr[j, c * 8 + p]
    W['LAr'] = Wla[0:20]
    W['LAi'] = Wla[20:40]
    # group head: 4 tiles at partition slots 32m..32m+19 (rows 0..115)
    blar = np.zeros((116,), dtype=np.float32)
    blai = np.zeros((116,), dtype=np.float32)
    for m in range(4):
        blar[32 * m:32 * m + 20] = g('la_br')
        blai[32 * m:32 * m + 20] = g('la_bi')
    bias['blar'] = blar
    bias['blai'] = blai

    # ---- FC: block-diagonal over the 4-tile group, K rows at 32m offsets
    fc1, fc2, fc3 = g('fc1_w'), g('fc2_w'), g('fc3_w')
    WFC1 = np.zeros((40, 116), dtype=np.float32)
    for m in range(4):
        WFC1[10 * m:10 * m + 10, 32 * m:32 * m + 20] = fc1
    W['FC1'] = WFC1
    W['FC2'] = np.kron(np.eye(4, dtype=np.float32), fc2)   # [40, 40]
    W['FC3'] = np.kron(np.eye(4, dtype=np.float32), fc3)   # [4, 40]
    bias['bfc1'] = np.tile(g('fc1_b'), 4)
    bias['bfc2'] = np.tile(g('fc2_b'), 4)
    bias['bfc3'] = np.tile(g('fc3_b'), 4)
    return W, bias


# ---------------------------------------------------------------------------
# Weight packing: one [128, cols] bf16 blob (lhsT blocks), one fp32 bias blob
# ---------------------------------------------------------------------------
def _pack(W, bias):
    import ml_dtypes
    cols = []
    index = {}

    def add(name, mat):  # mat [M, K] -> lhsT [K, M]
        lhsT = np.ascontiguousarray(mat.T)
        K, M = lhsT.shape
        off = sum(c.shape[1] for c in cols)
        buf = np.zeros((128, M), dtype=np.float32)
        buf[:K] = lhsT
        cols.append(buf)
        index[name] = (off, K, M)

    for k, Wk in enumerate(W['L1']):
        add(f'L1_{k}', Wk)
    for m, row in enumerate(W['L2']):
        for k, blk in enumerate(row):
            if np.any(blk):
                add(f'L2_{m}_{k}', blk)
    for m, blk in enumerate(W['SC1']):
        add(f'SC1_{m}', blk)
    for k, blk in enumerate(W['L3']):
        add(f'L3_{k}', blk)
    for m, blk in enumerate(W['L4']):
        add(f'L4_{m}', blk)
    for m, row in enumerate(W['SC2']):
        for k, blk in enumerate(row):
            add(f'SC2_{m}_{k}', blk)
    for nm in ('LAr', 'LAi', 'FC1', 'FC2', 'FC3'):
        add(nm, W[nm])
    wblob = np.concatenate(cols, axis=1).astype(ml_dtypes.bfloat16)

    bcols = []
    bindex = {}
    for nm, v in bias.items():
        buf = np.zeros((128,), dtype=np.float32)
        buf[:len(v)] = v
        bindex[nm] = (len(bcols), len(v))
        bcols.append(buf)
    bblob = np.stack(bcols, axis=1)  # [128, nb]
    return wblob, index, bblob, bindex


# ---------------------------------------------------------------------------
# Bass kernel
# ---------------------------------------------------------------------------
def _emit(nc, tens, windex, bindex):
    import concourse.mybir as mybir
    from concourse.tile import TileContext
    dt = mybir.dt
    AF = mybir.ActivationFunctionType
    OP = mybir.AluOpType
    x_d, w_d, b_d, id_d, out_d = tens
    BF = dt.bfloat16

    with TileContext(nc) as tc:
        with (
            tc.tile_pool(name="const", bufs=1) as cpool,
            tc.tile_pool(name="sb", bufs=2) as spool,
            tc.tile_pool(name="pa", bufs=1, space="PSUM") as ppa,
            tc.tile_pool(name="pd", bufs=1, space="PSUM") as ppd,
            tc.tile_pool(name="ps", bufs=1, space="PSUM") as pps,
        ):
            wsb = cpool.tile([128, w_d.shape[1]], BF, tag="wsb")
            nc.sync.dma_start(wsb, w_d[:, :])
            bsb = cpool.tile([128, b_d.shape[1]], dt.float32, tag="bsb")
            nc.sync.dma_start(bsb, b_d[:, :])
            ident = cpool.tile([128, 128], BF, tag="ident")
            nc.sync.dma_start(ident, id_d[:, :])
            # Pre-touch bsb on ACT and DVE: engine-sem waits elide per
            # engine, so later bias reads carry no DMA wait.
            scr = cpool.tile([128, 8], dt.float32, tag="scr")
            nc.scalar.activation(scr[:, 0:1], bsb[:, 0:1], AF.Identity)
            nc.vector.tensor_copy(scr[:, 1:2], bsb[:, 0:1])

            def sync_act(src):  # ACT absorbs a cross-engine wait
                nc.scalar.activation(scr[0:1, 2:3], src, AF.Identity)

            def sync_dve(src):  # DVE absorbs a cross-engine wait
                nc.vector.tensor_copy(scr[0:1, 3:4], src)


            def wap(name):
                off, K, M = windex[name]
                return wsb[0:K, off:off + M]

            def bap(name, P):
                col, _ = bindex[name]
                return bsb[0:P, col:col + 1]

            def matmul(out, name, rhs, start, stop):
                nc.tensor.matmul(out, wap(name), rhs, start=start, stop=stop)

            # Preload all of x into 8 never-reused SBUF chunks (8 tiles each).
            # No slot reuse -> the chunk DMAs carry no waits, and transposes
            # wait on at most one DMA semaphore.
            xchunks = []
            for c in range(8):
                xc = spool.tile([128, 8 * 264], BF, tag=f"xin{c}", bufs=1)
                src = x_d[c * 1024:(c + 1) * 1024, :].rearrange(
                    "(tt p) f -> p tt f", tt=8)
                # gpsimd SWDGE casts fp32 -> bf16 during the transfer
                nc.gpsimd.dma_start(xc.rearrange("p (tt f) -> p tt f", tt=8),
                                    src)
                xchunks.append(xc)

            for g0, G in GROUPS:
                # lr in cols 0:512, li in cols 512:1024; tile m of the group
                # occupies partitions 32m..32m+19 (gaps hold garbage)
                psig = pps.tile([128, 1024], dt.float32, tag="psig")
                if g0 == 0:
                    # zero once: the 12-row gaps between tile slots are never
                    # written by LA and would otherwise feed stale NaNs into
                    # sigmoid -> rho -> FC1 (0 * NaN = NaN).
                    nc.scalar.activation(psig, psig, AF.Is_finite)
                for j in range(G):
                    t = g0 + j
                    # ---- x tile from preloaded chunk; partition p holds
                    # samples 4p..4p+3 of the tile
                    xin = xchunks[t // 8]
                    tb = 264 * (t % 8)
                    # pa alloc holds both the x-transpose bank (cols 2048:2560)
                    # and the four L1 chunks (cols 0:2048)
                    pa = ppa.tile([128, 2560], dt.float32, tag="pa")
                    pt = pa[0:66, 2048:2560]
                    # 4 transposes (as normal matmuls: out = xin_slice.T @ I)
                    # into one PSUM bank: col u*128+j = sample 4j+u
                    for u in range(4):
                        nc.tensor.matmul(pt[:, u * 128:(u + 1) * 128],
                                         xin[:, tb + 66 * u:tb + 66 * (u + 1)],
                                         ident[:, :], start=True, stop=True)
                    x_t = spool.tile([66, NT], BF, tag="x_t")
                    nc.vector.tensor_copy(x_t, pt)

                    # ---- L1 -> tanh -> a1 [128, 2560]; chunk 4 (16 rows)
                    # reuses the pt bank once the x_t copy has drained it
                    for k in range(4):
                        matmul(pa[:, k * 512:(k + 1) * 512], f'L1_{k}', x_t,
                               True, True)
                    matmul(pa[0:16, 2048:2560], 'L1_4', x_t, True, True)
                    a1 = spool.tile([128, 2560], BF, tag="a1")
                    # rows 16-127 of the last bank are stale PSUM; tanh is
                    # bounded and those a1 columns are never read.
                    nc.scalar.activation(a1, pa, AF.Tanh, bias=bap('b1', 128))

                    def a1k(k):
                        if k < 4:
                            return a1[:, k * 512:(k + 1) * 512]
                        return a1[0:16, 2048:2560]

                    # ---- res1 conv2 + shortcut + tanh + add, e/o waves
                    s1 = []
                    for wave in range(2):  # 0: chunks e0,e1 ; 1: o0,o1
                        pw = ppa.tile([128, 2048], dt.float32, tag="pa")
                        pb = pw[:, 0:1024]
                        psc = pw[:, 1024:2048]
                        for h in range(2):
                            m = wave * 2 + h
                            ks = [k for k in range(5) if f'L2_{m}_{k}' in windex]
                            for i, k in enumerate(ks):
                                nm = f'L2_{m}_{k}'
                                off, K, M = windex[nm]
                                dst = pb[0:M, h * 512:(h + 1) * 512]
                                matmul(dst, nm, a1k(k), i == 0, i == len(ks) - 1)
                            matmul(psc[:, h * 512:(h + 1) * 512], f'SC1_{m}',
                                   x_t, True, True)
                        t2 = spool.tile([128, 1024], BF, tag="t2")
                        nc.scalar.activation(t2, pb, AF.Tanh, bias=bap('b2', 128))
                        s1w = spool.tile([128, 1024], BF, tag="s1")
                        nc.vector.scalar_tensor_tensor(
                            s1w, psc, bap('bsc1', 128), t2, OP.add, OP.add)
                        s1.append(s1w)
                    # pool1 on gpsimd (SBUF-only)
                    p1 = spool.tile([128, 1024], BF, tag="p1")
                    nc.vector.tensor_tensor(p1, s1[0], s1[1], OP.max)

                    # ---- res2
                    pd = ppd.tile([128, NT], dt.float32, tag="pd")
                    matmul(pd, 'L3_0', p1[:, 0:512], True, False)
                    matmul(pd, 'L3_1', p1[:, 512:1024], False, True)
                    a3 = spool.tile([128, NT], BF, tag="a3")
                    nc.scalar.activation(a3, pd, AF.Tanh, bias=bap('b3', 128))

                    s2 = []
                    for m in range(2):
                        pe = ppd.tile([64, NT], dt.float32, tag="pd")
                        matmul(pe, f'L4_{m}', a3, True, True)
                        t4 = spool.tile([64, NT], BF, tag="t4")
                        nc.scalar.activation(t4, pe, AF.Tanh, bias=bap('b4', 64))
                        pf = ppd.tile([64, NT], dt.float32, tag="pd")
                        matmul(pf, f'SC2_{m}_0', p1[:, 0:512], True, False)
                        matmul(pf, f'SC2_{m}_1', p1[:, 512:1024], False, True)
                        s2w = spool.tile([64, NT], BF, tag="s2")
                        nc.vector.scalar_tensor_tensor(
                            s2w, pf, bap('bsc2', 64), t4, OP.add, OP.add)
                        s2.append(s2w)
                    p2 = spool.tile([64, NT], BF, tag="p2")
                    nc.vector.tensor_tensor(p2, s2[0], s2[1], OP.max)

                    # ---- complex linear into the group sigma psum
                    nc.tensor.matmul(psig[32 * j:32 * j + 20, 0:512],
                                     wap('LAr'), p2, start=True, stop=True,
                                     tile_position=(0, 32 * j))
                    nc.tensor.matmul(psig[32 * j:32 * j + 20, 512:1024],
                                     wap('LAi'), p2, start=True, stop=True,
                                     tile_position=(0, 32 * j))

                # ---- group head: sigmoid, arctan(si/sr), FC
                sg = spool.tile([116, 1024], BF, tag="sg")
                nc.scalar.activation(sg[:, 0:512], psig[0:116, 0:512],
                                     AF.Sigmoid, bias=bap('blar', 116))
                nc.scalar.activation(sg[:, 512:1024], psig[0:116, 512:1024],
                                     AF.Sigmoid, bias=bap('blai', 116))
                rc = spool.tile([116, NT], BF, tag="rc")
                with nc.allow_low_precision(reason="bf16 head, tol 2e-2"):
                    nc.vector.reciprocal(rc, sg[:, 0:512])
                q = spool.tile([116, NT], BF, tag="q")
                nc.vector.tensor_tensor(q, sg[:, 512:1024], rc, OP.mult)
                rho = spool.tile([116, NT], BF, tag="rho")
                nc.scalar.activation(rho, q, AF.Arctan)

                ph1 = ppd.tile([40, NT], dt.float32, tag="pd")
                matmul(ph1, 'FC1', rho, True, True)
                h1 = spool.tile([40, NT], BF, tag="h1")
                nc.scalar.activation(h1, ph1, AF.Tanh, bias=bap('bfc1', 40))
                ph2 = ppd.tile([40, NT], dt.float32, tag="pd")
                matmul(ph2, 'FC2', h1, True, True)
                h2 = spool.tile([40, NT], BF, tag="h2")
                nc.scalar.activation(h2, ph2, AF.Tanh, bias=bap('bfc2', 40))
                ph3 = ppd.tile([4, NT], dt.float32, tag="pd")
                matmul(ph3, 'FC3', h2, True, True)
                # bufs=16: one slot per group, never recycled -> the out DMA
                # and this ACT each carry a single wait
                ot = spool.tile([4, NT], dt.float32, tag="ot", bufs=16)
                nc.scalar.activation(ot, ph3, AF.Identity, bias=bap('bfc3', 4))
                # col u*128+jj <-> sample 4*jj+u: un-permuted on the host
                nc.sync.dma_start(out_d[g0:g0 + 4, :], ot)
    return nc


def _numpy_forward(inp):
    """Reference fallback in numpy (slow but exact)."""
    g = lambda n: np.asarray(inp[n], dtype=np.float32)

    def conv(x, w, b, pad):
        Bx, Ci, L = x.shape
        Co = w.shape[0]
        xp = np.pad(x, ((0, 0), (0, 0), (pad, pad)))
        Lo = L
        if pad == 0:
            Lo = L - w.shape[2] + 1
        out = np.zeros((Bx, Co, Lo), dtype=np.float32)
        for k in range(w.shape[2]):
            out += np.einsum('bil,oi->bol', xp[:, :, k:k + Lo], w[:, :, k])
        return out + b[None, :, None]

    def cconv(xr, xi, wr, wi, br, bi, pad):
        ar = conv(xr, wr, br, pad) - conv(xi, wi, bi, pad)
        ai = conv(xr, wi, bi, pad) + conv(xi, wr, br, pad)
        return ar, ai

    x = g('x')
    xr, xi = x[:, 0:1, :], x[:, 1:2, :]
    ar, ai = cconv(xr, xi, g('r1c1_wr'), g('r1c1_wi'), g('r1c1_br'), g('r1c1_bi'), 1)
    ar, ai = np.tanh(ar), np.tanh(ai)
    ar, ai = cconv(ar, ai, g('r1c2_wr'), g('r1c2_wi'), g('r1c2_br'), g('r1c2_bi'), 1)
    ar, ai = np.tanh(ar), np.tanh(ai)
    sr, si = cconv(xr, xi, g('r1sc_wr'), g('r1sc_wi'), g('r1sc_br'), g('r1sc_bi'), 0)
    ar, ai = ar + sr, ai + si
    pool = lambda v: v[:, :, :(v.shape[2] // 2) * 2].reshape(
        v.shape[0], v.shape[1], -1, 2).max(-1)
    ar, ai = pool(ar), pool(ai)
    br_, bi_ = ar, ai
    ar, ai = cconv(br_, bi_, g('r2c1_wr'), g('r2c1_wi'), g('r2c1_br'), g('r2c1_bi'), 1)
    ar, ai = np.tanh(ar), np.tanh(ai)
    ar, ai = cconv(ar, ai, g('r2c2_wr'), g('r2c2_wi'), g('r2c2_br'), g('r2c2_bi'), 1)
    ar, ai = np.tanh(ar), np.tanh(ai)
    sr, si = cconv(br_, bi_, g('r2sc_wr'), g('r2sc_wi'), g('r2sc_br'), g('r2sc_bi'), 0)
    ar, ai = pool(ar + sr), pool(ai + si)
    Bx = ar.shape[0]
    cr, ci = ar.reshape(Bx, -1), ai.reshape(Bx, -1)
    lr = cr @ g('la_wr').T - ci @ g('la_wi').T + g('la_br')
    li = cr @ g('la_wi').T + ci @ g('la_wr').T + g('la_bi')
    sgm = lambda v: 1.0 / (1.0 + np.exp(-v))
    rho = np.arctan(sgm(li) / sgm(lr))
    h = np.tanh(rho @ g('fc1_w').T + g('fc1_b'))
    h = np.tanh(h @ g('fc2_w').T + g('fc2_b'))
    return (h @ g('fc3_w').T + g('fc3_b'))[:, 0].astype(np.float32)


_CACHE = {}


def kernel(**inputs):
    try:
        return _kernel_bass(**inputs)
    except Exception as e:
        import traceback
        traceback.print_exc()
        print("BASS PATH FAILED -> numpy fallback:", e)
        return _numpy_forward(inputs)


def _kernel_bass(**inputs):
    from concourse import bass_utils

    W, bias = _build_host(inputs)
    wblob, windex, bblob, bindex = _pack(W, bias)

    key = (wblob.shape[1], bblob.shape[1])
    if key not in _CACHE:
        import concourse.bacc as bacc
        import concourse.mybir as mybir
        dt = mybir.dt
        nc = bacc.Bacc()
        x_d = nc.dram_tensor("x", [BC // 4, 264], dt.float32,
                             kind="ExternalInput")
        w_d = nc.dram_tensor("wblob", [128, wblob.shape[1]], dt.bfloat16,
                             kind="ExternalInput")
        b_d = nc.dram_tensor("bblob", [128, bblob.shape[1]], dt.float32,
                             kind="ExternalInput")
        id_d = nc.dram_tensor("ident", [128, 128], dt.bfloat16,
                              kind="ExternalInput")
        out_d = nc.dram_tensor("out", [NTILES, NT], dt.float32,
                               kind="ExternalOutput")
        nc = _emit(nc, (x_d, w_d, b_d, id_d, out_d), windex, bindex)
        nc.finalize()
        _CACHE[key] = nc
    nc = _CACHE[key]

    x = np.ascontiguousarray(
        np.asarray(inputs['x'], dtype=np.float32).reshape(B // 4, 264))
    import ml_dtypes
    ident = np.eye(128, dtype=ml_dtypes.bfloat16)
    in_maps = []
    for c in range(NCORES):
        in_maps.append({
            "x": x[c * (BC // 4):(c + 1) * (BC // 4)],
            "wblob": wblob,
            "bblob": bblob,
            "ident": ident,
        })
    res = bass_utils.run_bass_kernel_spmd(nc, in_maps, list(range(NCORES)))
    # kernel col u*128+jj within a tile holds sample 4*jj+u -> un-permute
    outs = [np.asarray(r["out"]).reshape(NTILES, 4, 128).transpose(0, 2, 1)
            .reshape(BC) for r in res.results]
    return np.concatenate(outs).astype(np.float32)


if __name__ == "__main__":
    print("host-build smoke test")
